# revision 18
# baseline (speedup 1.0000x reference)
"""DiM block (adaLN + Mamba selective scan + MLP) on 8 Trainium2 NeuronCores.

Sharding: core c = 2*b + i serves batch b; within a pair, core i owns
d_inner half i for the FULL sequence (the scan is fully local) and time
half i for the back-end (out_proj/LN2/MLP).
Pair collectives: AllReduce of x_proj partials (2 chunks of [96,1024])
and AllGather of gated-y (2 chunks of [P,4,1024] bf16 per core).

v2: scan restructured to TB=512 blocks with SBUF-cached B/C broadcasts
(was 512 redundant DMA broadcasts), ym split across DVE+Pool, LN mults
on Pool, hsm kept in SBUF (no DRAM round trip), bf16 adaLN weight,
chunked collectives to shrink dependency stalls.
"""
import sys
import numpy as np

sys.path.insert(0, "/opt/trn_rl_repo")

import concourse.bass as bass
import concourse.mybir as mybir
import concourse.tile as tile
from concourse import bacc
from concourse.bass_utils import run_bass_kernel_spmd
from concourse.masks import make_identity

import ml_dtypes

F32 = mybir.dt.float32
F32R = mybir.dt.float32r
BF16 = mybir.dt.bfloat16
AF = mybir.ActivationFunctionType
OP = mybir.AluOpType

P = 128
B, L, DIM = 4, 2048, 1024
D_STATE, D_CONV = 16, 4
D_INNER = 2048
DH = D_INNER // 2
DT_RANK = 64
MLP_HID = 4 * DIM
T = L
TH = L // 2
NC = 8
KD = DIM // P      # 8
KDH = DH // P      # 8
KH = MLP_HID // P  # 32
NT = T // 512      # 4
NTH = TH // 512    # 2
TB = 512           # scan time-block
NBT = T // TB      # 4
RG = [[0, 1], [2, 3], [4, 5], [6, 7]]

YM_POOL_FROM = 3    # states n >= this run ym on Pool (gpsimd), rest on DVE
SCAN_POOL_FROM = 16  # states n >= this run the scan itself on Pool

_CACHE = {}


def _build():
    nc = bacc.Bacc("TRN2", target_bir_lowering=False, debug=False, num_devices=NC)
    ein = lambda n, s, d=F32: nc.dram_tensor(n, s, d, kind="ExternalInput")
    hsT = ein("hsT", (DIM, T), BF16)
    cond_c = ein("cond_c", (P, KD))
    w_ada = ein("w_ada", (DIM, 6 * DIM), BF16)
    ada_b = ein("ada_b", (1, 6 * DIM))
    nrm_w = ein("nrm_w", (P, KD))
    nrm_b = ein("nrm_b", (P, KD))
    w_inx = ein("w_inx", (DIM, DH), BF16)
    w_inz = ein("w_inz", (DIM, DH), BF16)
    conv_wt = ein("conv_wt", (P, KDH * D_CONV))
    conv_bt = ein("conv_bt", (P, KDH))
    w_xp = ein("w_xp", (DH, 96), BF16)
    w_dt = ein("w_dt", (P, DH), F32R)
    dt_bt = ein("dt_bt", (P, KDH))
    A_sc = ein("A_sc", (P, KDH * D_STATE))
    Dp_c = ein("Dp_c", (P, KDH))
    w_out = ein("w_out", (D_INNER, DIM), BF16)   # rows: my d-half then partner's
    w_fc1 = ein("w_fc1", (DIM, MLP_HID), BF16)
    fc1_bt = ein("fc1_bt", (P, KH))
    w_fc2 = ein("w_fc2", (MLP_HID, DIM), BF16)
    fc2_bt = ein("fc2_bt", (P, KD))
    pairsel = ein("pairsel", (P, 2))
    out_hs = nc.dram_tensor("out_hs", (DIM, TH), F32, kind="ExternalOutput")

    with tile.TileContext(nc) as tc, nc.allow_low_precision(
            reason="bf16 value path; rel-err budget 2e-2"):
        import contextlib
        ctx = contextlib.ExitStack()
        sing = ctx.enter_context(tc.tile_pool(name="sing", bufs=1))
        ws = ctx.enter_context(tc.tile_pool(name="ws", bufs=2))
        wk1 = ctx.enter_context(tc.tile_pool(name="wk1", bufs=1))
        wk2 = ctx.enter_context(tc.tile_pool(name="wk2", bufs=2))
        ga = ctx.enter_context(tc.tile_pool(name="ga", bufs=3))
        ps = ctx.enter_context(tc.tile_pool(name="ps", bufs=4, space="PSUM"))
        psy = ctx.enter_context(tc.tile_pool(name="psy", bufs=1, space="PSUM"))
        dpool = ctx.enter_context(tc.tile_pool(name="dram", bufs=1, space="DRAM"))

        # DRAM scratch
        md_dram = dpool.tile([1, 6 * DIM], F32)
        cc1_in0 = dpool.tile([96, TH], F32)
        cc1_out0 = dpool.tile([96, TH], F32)
        cc1_in1 = dpool.tile([96, TH], F32)
        cc1_out1 = dpool.tile([96, TH], F32)
        bc_dram = dpool.tile([2 * D_STATE, T], BF16)
        zs_dram = dpool.tile([P, KDH, T], BF16)
        cc2_in = dpool.tile([P, KDH, T], BF16)        # full gated-y spill
        cc2s_in0 = dpool.tile([P, 4, TH], BF16)       # partner-needed quadrants
        cc2s_in1 = dpool.tile([P, 4, TH], BF16)
        cc2s_out0 = dpool.tile([2, P, 4, TH], BF16)
        cc2s_out1 = dpool.tile([2, P, 4, TH], BF16)

        # ---- small persistents ----
        psel_sb = sing.tile([P, 2], F32)
        nc.sync.dma_start(psel_sb[:], pairsel[:])
        cond_sb = sing.tile([P, KD], F32)
        nc.sync.dma_start(cond_sb[:], cond_c[:])
        scond = sing.tile([P, KD], BF16)
        nc.scalar.activation(scond[:], cond_sb[:], AF.Silu)
        nwb_sb = sing.tile([P, 2 * KD], F32)
        nc.sync.dma_start(nwb_sb[:, 0:KD], nrm_w[:])
        nc.sync.dma_start(nwb_sb[:, KD:2 * KD], nrm_b[:])
        cwb_sb = sing.tile([P, KDH * D_CONV + KDH], F32)
        nc.sync.dma_start(cwb_sb[:, 0:KDH * D_CONV], conv_wt[:, :])
        nc.sync.dma_start(cwb_sb[:, KDH * D_CONV:], conv_bt[:])
        dtb_sb = sing.tile([P, KDH], F32)
        nc.sync.dma_start(dtb_sb[:], dt_bt[:])
        Asc_sb = sing.tile([P, KDH * D_STATE], F32)
        nc.sync.dma_start(Asc_sb[:], A_sc[:])
        Dp_sb = sing.tile([P, KDH], F32)
        nc.sync.dma_start(Dp_sb[:], Dp_c[:])
        fb_sb = sing.tile([P, KH + KD], F32)
        nc.sync.dma_start(fb_sb[:, 0:KH], fc1_bt[:])
        nc.sync.dma_start(fb_sb[:, KH:], fc2_bt[:])
        ident = sing.tile([P, P], BF16)
        make_identity(nc, ident[:])
        ones_bf = sing.tile([P, 1], BF16)
        nc.vector.memset(ones_bf[:], 1.0)
        ones_row = sing.tile([1, P], F32R)
        nc.vector.memset(ones_row[:].bitcast(F32), 1.0)
        eps_sb = sing.tile([1, 2], F32)
        nc.vector.memset(eps_sb[:, 0:1], 1e-5)
        nc.vector.memset(eps_sb[:, 1:2], 1e-6)
        one_col = sing.tile([P, 1], F32)
        nc.vector.memset(one_col[:], 1.0)
        ones_colR = sing.tile([P, 1], F32R)
        nc.vector.memset(ones_colR[:].bitcast(F32), 1.0)
        hcol = sing.tile([P, KDH * D_STATE], F32)   # scan carry per (mj, n)

        # ---- adaLN mods ----
        for fj in range(24):
            wada = ws.tile([P, KD, 256], BF16, tag="ws")
            nc.sync.dma_start(wada[:], w_ada[:, fj * 256:(fj + 1) * 256]
                              .rearrange("(k p) f -> p k f", p=P))
            mp = psy.tile([1, 256], F32, tag="rowA")
            for kj in range(KD):
                nc.tensor.matmul(mp[:], scond[:, kj:kj + 1], wada[:, kj],
                                 start=(kj == 0), stop=(kj == KD - 1))
            adab_c = ga.tile([1, 256], F32, tag="gR")
            nc.sync.dma_start(adab_c[:], ada_b[0:1, fj * 256:(fj + 1) * 256])
            mrow = ga.tile([1, 256], F32, tag="gR")
            nc.vector.tensor_tensor(mrow[:], mp[:], adab_c[:], OP.add)
            nc.sync.dma_start(md_dram[0:1, fj * 256:(fj + 1) * 256], mrow[:])
        modsT = sing.tile([P, 6 * KD], F32)
        nc.sync.dma_start(modsT[:], md_dram[0:1, :].rearrange("q (c p) -> (q p) c", p=P))
        sh_msa, sc_msa, gt_msa, sh_mlp, sc_mlp, gt_mlp = (
            modsT[:, k * KD:(k + 1) * KD] for k in range(6))
        a1 = sing.tile([P, 3 * KD], F32)   # [a1 | c1 | a2]
        nc.vector.tensor_scalar(a1[:, 2 * KD:3 * KD], sc_mlp, 1.0, None, OP.add)
        tmp8 = sing.tile([P, KD], F32)
        nc.vector.tensor_scalar(tmp8[:], sc_msa, 1.0, None, OP.add)
        nc.vector.tensor_tensor(a1[:, 0:KD], nwb_sb[:, 0:KD], tmp8[:], OP.mult)
        nc.vector.tensor_tensor(a1[:, KD:2 * KD], nwb_sb[:, KD:2 * KD], tmp8[:], OP.mult)
        nc.vector.tensor_tensor(a1[:, KD:2 * KD], a1[:, KD:2 * KD], sh_msa, OP.add)

        # ---- LN1 + modulate, fused per 512-tile (emitted per time-half) ----
        # x_in half 0 shares the BCC region (dead before scan tb0 fills it);
        # half 1 shares a 16KB region with h2_bf (written post-scan).
        x_in0 = sing.tile([P, KD, TH], BF16, tag="BCC")
        x_in1 = sing.tile([P, KD, TH], BF16, tag="XH1")
        x_inh = [x_in0, x_in1]

        def ln1_tile(ft):
            fs = slice(ft * 512, ft * 512 + 512)
            hst = ws.tile([P, KD, 512], BF16, tag="ws")
            nc.sync.dma_start(hst[:], hsT[:, fs].rearrange("(k p) t -> p k t", p=P))
            sp = psy.tile([1, 512], F32, tag="rowA")
            qp = psy.tile([1, 512], F32, tag="rowB")
            for kj in range(KD):
                nc.tensor.matmul(sp[:], ones_bf[:], hst[:, kj],
                                 start=(kj == 0), stop=(kj == KD - 1))
            for kj in range(KD):
                sq = ga.tile([P, 512], BF16, tag="gA")
                nc.scalar.activation(sq[:], hst[:, kj], AF.Square)
                nc.tensor.matmul(qp[:], ones_bf[:], sq[:],
                                 start=(kj == 0), stop=(kj == KD - 1))
            mu = ga.tile([1, 512], F32R, tag="gR")
            nc.vector.tensor_scalar(mu[:], sp[:], 1.0 / DIM, None, OP.mult)
            mu_pl = psy.tile([P, 512], F32, tag="rowA")
            nc.tensor.matmul(mu_pl[:], ones_row[:], mu[:], start=True, stop=True)
            mq = ga.tile([1, 512], F32, tag="gR")
            nc.vector.tensor_tensor(mq[:], mu[:], mu[:], OP.mult)
            vt = ga.tile([1, 512], F32, tag="gR")
            nc.vector.scalar_tensor_tensor(vt[:], qp[:], 1.0 / DIM, mq[:],
                                           OP.mult, OP.subtract)
            nc.scalar.activation(vt[:], vt[:], AF.Sqrt, bias=eps_sb[:, 0:1])
            rs = ga.tile([1, 512], F32R, tag="gR")
            nc.vector.reciprocal(rs[:], vt[:])
            rs_pl = psy.tile([P, 512], F32, tag="rowB")
            nc.tensor.matmul(rs_pl[:], ones_row[:], rs[:], start=True, stop=True)
            rs_sb = ga.tile([P, 512], F32, tag="gA")
            nc.scalar.activation(rs_sb[:], rs_pl[:], AF.Copy)
            for kj in range(KD):
                t1 = ga.tile([P, 512], F32, tag="gA")
                nc.vector.tensor_tensor(t1[:], hst[:, kj], mu_pl[:], OP.subtract)
                nc.gpsimd.tensor_tensor(t1[:], t1[:], rs_sb[:], OP.mult)
                nc.vector.tensor_scalar(
                    x_inh[ft // 2][:, kj, (ft % 2) * 512:(ft % 2) * 512 + 512],
                    t1[:], a1[:, kj:kj + 1], a1[:, KD + kj:KD + kj + 1],
                    OP.mult, OP.add)

        # ---- front-end per time-half: in_proj x + conv -> x_proj -> CC1
        #      -> in_proj z -> dt/u.  Lets scan(tb0) start while half 1
        #      is still in flight. ----
        xcv_bf = sing.tile([P, KDH, T], BF16, tag="XCV4")
        wxp_sb = sing.tile([P, KDH, 96], BF16)
        nc.sync.dma_start(wxp_sb[:], w_xp[:, :].rearrange("(k p) m -> p k m", p=P))
        xhalo = sing.tile([P, KDH * (D_CONV - 1)], BF16)
        dtlow = sing.tile([P, T], F32R, tag="DTL8")
        nc.vector.memset(dtlow[:].bitcast(F32), 0.0)
        wdt_sb = sing.tile([P, DH], F32R)
        nc.sync.dma_start(wdt_sb[:], w_dt[:])
        dtu = sing.tile([P, 2 * KDH, T], BF16, tag="BIG8")   # [0:8]=dt, [8:16]=u
        for hf in range(2):
            hs_ = slice(hf * TH, (hf + 1) * TH)
            cc1_in = cc1_in0 if hf == 0 else cc1_in1
            cc1_out = cc1_out0 if hf == 0 else cc1_out1
            for fq in range(2):
                ln1_tile(hf * 2 + fq)
            for mj in range(KDH):
                wx = ws.tile([P, KD, P], BF16, tag="ws")
                nc.sync.dma_start(wx[:], w_inx[:, mj * P:(mj + 1) * P]
                                  .rearrange("(k p) m -> p k m", p=P))
                xpad = wk1.tile([P, D_CONV - 1 + TH], BF16, tag="xpad")
                hsl = slice(mj * (D_CONV - 1), (mj + 1) * (D_CONV - 1))
                if hf == 0:
                    nc.vector.memset(xpad[:, 0:D_CONV - 1], 0.0)
                else:
                    nc.vector.tensor_copy(xpad[:, 0:D_CONV - 1], xhalo[:, hsl])
                for fq in range(2):
                    ft = hf * 2 + fq
                    fs = slice(ft * 512, ft * 512 + 512)
                    px = ps.tile([P, 512], F32, tag="mm")
                    xfs = slice(fq * 512, fq * 512 + 512)
                    for kj in range(KD):
                        nc.tensor.matmul(px[:], wx[:, kj], x_inh[hf][:, kj, xfs],
                                         start=(kj == 0), stop=(kj == KD - 1))
                    nc.scalar.activation(
                        xpad[:, D_CONV - 1 + fq * 512:D_CONV - 1 + (fq + 1) * 512],
                        px[:], AF.Copy)
                if hf == 0:
                    nc.vector.tensor_copy(xhalo[:, hsl], xpad[:, TH:TH + D_CONV - 1])
                for th in range(2):
                    base = th * 512
                    cvt = wk1.tile([P, 512], BF16, tag="cvt")
                    nc.vector.tensor_scalar(cvt[:], xpad[:, base:base + 512],
                                            cwb_sb[:, mj * D_CONV:mj * D_CONV + 1],
                                            None, OP.mult)
                    for j in range(1, D_CONV):
                        nc.vector.scalar_tensor_tensor(
                            cvt[:], xpad[:, base + j:base + j + 512],
                            cwb_sb[:, mj * D_CONV + j:mj * D_CONV + j + 1], cvt[:],
                            OP.mult, OP.add)
                    nc.scalar.activation(
                        xcv_bf[:, mj, hf * TH + base:hf * TH + base + 512], cvt[:],
                        AF.Silu,
                        bias=cwb_sb[:, KDH * D_CONV + mj:KDH * D_CONV + mj + 1])
            # x_proj partials for this half -> AllReduce
            for fq in range(2):
                ft = hf * 2 + fq
                fs = slice(ft * 512, ft * 512 + 512)
                xdp = ps.tile([96, 512], F32, tag="mm")
                for mj in range(KDH):
                    nc.tensor.matmul(xdp[:], wxp_sb[:, mj], xcv_bf[:, mj, fs],
                                     start=(mj == 0), stop=(mj == KDH - 1))
                xdc = ga.tile([96, 512], F32, tag="gA")
                nc.scalar.activation(xdc[:], xdp[:], AF.Copy)
                nc.sync.dma_start(cc1_in[:, fq * 512:(fq + 1) * 512], xdc[:])
            nc.gpsimd.collective_compute("AllReduce", OP.add, ins=[cc1_in[:]],
                                         outs=[cc1_out[:]], replica_groups=RG)
            # in_proj z for this half (overlaps the AllReduce)
            for mj in range(KDH):
                wz = ws.tile([P, KD, P], BF16, tag="ws")
                nc.sync.dma_start(wz[:], w_inz[:, mj * P:(mj + 1) * P]
                                  .rearrange("(k p) m -> p k m", p=P))
                for fq in range(2):
                    ft = hf * 2 + fq
                    fs = slice(ft * 512, ft * 512 + 512)
                    pz = ps.tile([P, 512], F32, tag="mm")
                    xfs = slice(fq * 512, fq * 512 + 512)
                    for kj in range(KD):
                        nc.tensor.matmul(pz[:], wz[:, kj], x_inh[hf][:, kj, xfs],
                                         start=(kj == 0), stop=(kj == KD - 1))
                    zs = ga.tile([P, 512], BF16, tag="gA")
                    nc.scalar.activation(zs[:], pz[:], AF.Silu)
                    nc.sync.dma_start(zs_dram[:, mj, fs], zs[:])
            nc.gpsimd.dma_start(dtlow[0:DT_RANK, hs_].bitcast(F32),
                                cc1_out[0:DT_RANK, :])
            bc_bf = sing.tile([2 * D_STATE, TH], BF16, tag="BCW")
            nc.gpsimd.dma_start(bc_bf[:], cc1_out[DT_RANK:96, :])
            nc.sync.dma_start(bc_dram[:, hs_], bc_bf[:])
            # dt (softplus via exp->ln1p) + u for this half
            for mj in range(KDH):
                for fq in range(2):
                    ft = hf * 2 + fq
                    fs = slice(ft * 512, ft * 512 + 512)
                    pdt = ps.tile([P, 512], F32, tag="mm")
                    nc.tensor.matmul(pdt[:], wdt_sb[:, mj * P:(mj + 1) * P],
                                     dtlow[:, fs], start=True, stop=True)
                    edt = ga.tile([P, 512], F32, tag="gA")
                    nc.scalar.activation(edt[:], pdt[:], AF.Exp,
                                         bias=dtb_sb[:, mj:mj + 1])
                    nc.scalar.activation(dtu[:, mj, fs], edt[:], AF.Ln,
                                         bias=one_col[:])
                nc.vector.tensor_tensor(dtu[:, KDH + mj, hs_], dtu[:, mj, hs_],
                                        xcv_bf[:, mj, hs_], OP.mult)

        # ---- phase 2b: scan + y + gate (TB=512 blocks, SBUF B/C cache).
        #      At the last block, CC2 prep + chunked AllGathers are
        #      interleaved with the remaining channel blocks. ----
        def cc2_prep(mj):
            # rank0 partner home = [TH:T]; rank1 partner home = [0:TH]
            g, mi = mj // 4, mj % 4
            cc2s_in = cc2s_in0 if g == 0 else cc2s_in1
            for q in range(NTH):
                qs = slice(q * 512, q * 512 + 512)
                ta = ga.tile([P, 512], BF16, tag="gA")
                nc.sync.dma_start(ta[:],
                                  cc2_in[:, mj, TH + q * 512:TH + q * 512 + 512])
                tb_ = ga.tile([P, 512], BF16, tag="gA")
                nc.sync.dma_start(tb_[:], cc2_in[:, mj, qs])
                nc.vector.tensor_scalar(ta[:], ta[:], psel_sb[:, 0:1],
                                        None, OP.mult)
                tc_ = ga.tile([P, 512], BF16, tag="gA")
                nc.vector.scalar_tensor_tensor(tc_[:], tb_[:], psel_sb[:, 1:2],
                                               ta[:], OP.mult, OP.add)
                nc.sync.dma_start(cc2s_in[:, mi, qs], tc_[:])

        for tb in range(NBT):
            ts_ = slice(tb * TB, (tb + 1) * TB)
            bcc = sing.tile([P, 2 * D_STATE, TB], BF16, tag="BCC")
            for n in range(2 * D_STATE):
                nc.sync.dma_start(bcc[:, n], bc_dram[n:n + 1, ts_]
                                  .partition_broadcast(P))
            for mj in range(KDH):
                ypsum = psy.tile([P, TB], F32, tag=f"y{mj % 2}", name=f"y{mj % 2}")
                dpl = ga.tile([P, TB], BF16, tag="gA")
                nc.scalar.activation(dpl[:], xcv_bf[:, mj, ts_], AF.Copy,
                                     scale=Dp_sb[:, mj:mj + 1])
                nc.tensor.matmul(ypsum[:], ident[:], dpl[:], start=True, stop=False)
                for n in range(D_STATE):
                    ci = mj * D_STATE + n
                    da = wk2.tile([P, TB], F32, tag="da")
                    nc.scalar.activation(
                        da[:], dtu[:, mj, ts_], AF.Exp,
                        scale=Asc_sb[:, ci:ci + 1])
                    bin_ = wk2.tile([P, TB], BF16, tag="bin")
                    nc.vector.tensor_tensor(bin_[:], dtu[:, KDH + mj, ts_],
                                            bcc[:, n], OP.mult)
                    h = wk2.tile([P, TB], F32, tag="h")
                    init = 0.0 if tb == 0 else hcol[:, ci:ci + 1]
                    seng = nc.gpsimd if n >= SCAN_POOL_FROM else nc.vector
                    seng.tensor_tensor_scan(h[:], da[:], bin_[:], init,
                                            OP.mult, OP.add)
                    if tb < NBT - 1:
                        nc.vector.tensor_copy(hcol[:, ci:ci + 1], h[:, TB - 1:TB])
                    ym = wk2.tile([P, TB], BF16, tag="ym")
                    eng = nc.gpsimd if n >= YM_POOL_FROM else nc.vector
                    eng.tensor_tensor(ym[:], h[:], bcc[:, D_STATE + n], OP.mult)
                    nc.tensor.matmul(ypsum[:], ident[:], ym[:],
                                     start=False, stop=(n == D_STATE - 1))
                zst = ga.tile([P, TB], BF16, tag="gA")
                nc.sync.dma_start(zst[:], zs_dram[:, mj, ts_])
                yg = ga.tile([P, TB], BF16, tag="gA")
                nc.vector.tensor_tensor(yg[:], ypsum[:], zst[:], OP.mult)
                nc.sync.dma_start(cc2_in[:, mj, ts_], yg[:])
                if tb == NBT - 1:
                    cc2_prep(mj)
        nc.gpsimd.collective_compute(
            "AllGather", OP.bypass, ins=[cc2s_in0[:]],
            outs=[cc2s_out0[:]], replica_groups=RG)
        nc.gpsimd.collective_compute(
            "AllGather", OP.bypass, ins=[cc2s_in1[:]],
            outs=[cc2s_out1[:]], replica_groups=RG)

        # ---- phase 3a: blend quadrants by pair rank ----
        # ygx[0:8] = my d-half on my home half; ygx[8:16] = partner's d-half.
        ygx = sing.tile([P, 2 * KDH, TH], BF16, tag="XCV4")
        for mj in range(KDH):
            for q in range(NTH):
                qs = slice(q * 512, q * 512 + 512)
                # own d-half on my home: rank0 -> yg[0:TH], rank1 -> yg[TH:T]
                ta = wk2.tile([P, 512], BF16, tag="bin")
                tb_ = wk2.tile([P, 512], BF16, tag="ym")
                nc.sync.dma_start(ta[:], cc2_in[:, mj, q * 512:q * 512 + 512])
                nc.sync.dma_start(tb_[:], cc2_in[:, mj,
                                               TH + q * 512:TH + q * 512 + 512])
                nc.vector.tensor_scalar(ta[:], ta[:], psel_sb[:, 0:1],
                                        None, OP.mult)
                nc.vector.scalar_tensor_tensor(ygx[:, mj, qs], tb_[:],
                                               psel_sb[:, 1:2], ta[:],
                                               OP.mult, OP.add)
        for mj in range(KDH):
            cc2s_out = cc2s_out0 if mj < 4 else cc2s_out1
            mi = mj % 4
            for q in range(NTH):
                qs = slice(q * 512, q * 512 + 512)
                # partner's d-half on my home = partner's sent row:
                # rank0 -> cc2s_out[1], rank1 -> cc2s_out[0]
                ta2 = wk2.tile([P, 512], BF16, tag="bin")
                tb2 = wk2.tile([P, 512], BF16, tag="ym")
                nc.sync.dma_start(ta2[:], cc2s_out[1, :, mi, qs])
                nc.sync.dma_start(tb2[:], cc2s_out[0, :, mi, qs])
                nc.vector.tensor_scalar(ta2[:], ta2[:], psel_sb[:, 0:1],
                                        None, OP.mult)
                nc.vector.scalar_tensor_tensor(ygx[:, KDH + mj, qs], tb2[:],
                                               psel_sb[:, 1:2], ta2[:],
                                               OP.mult, OP.add)

        # ---- phase 3b: out_proj + gate_msa -> SBUF hsm ----
        hsm_sb = sing.tile([P, KD, TH], F32R, tag="BCC")
        for oj in range(KD):
            wo = ws.tile([P, 2 * KDH, P], BF16, tag="ws")
            nc.sync.dma_start(wo[:], w_out[:, oj * P:(oj + 1) * P]
                              .rearrange("(k p) m -> p k m", p=P))
            for ft in range(NTH):
                fs = slice(ft * 512, ft * 512 + 512)
                po = ps.tile([P, 512], F32, tag="mm")
                for kj in range(2 * KDH):
                    nc.tensor.matmul(po[:], wo[:, kj], ygx[:, kj, fs],
                                     start=(kj == 0), stop=(kj == 2 * KDH - 1))
                nc.vector.tensor_scalar(hsm_sb[:, oj, fs], po[:],
                                        gt_msa[:, oj:oj + 1], None, OP.mult)

        # ---- LN2 + modulate (fused per 512-tile, SBUF source) ----
        h2_bf = sing.tile([P, KD, TH], BF16, tag="XH1")
        for ft in range(NTH):
            fs = slice(ft * 512, ft * 512 + 512)
            sp = psy.tile([1, 512], F32, tag="rowA")
            qp = psy.tile([1, 512], F32, tag="rowB")
            for kj in range(KD):
                nc.tensor.matmul(sp[:], ones_colR[:], hsm_sb[:, kj, fs],
                                 start=(kj == 0), stop=(kj == KD - 1))
            for kj in range(KD):
                sq = ga.tile([P, 512], BF16, tag="gA")
                nc.scalar.activation(sq[:], hsm_sb[:, kj, fs], AF.Square)
                nc.tensor.matmul(qp[:], ones_bf[:], sq[:],
                                 start=(kj == 0), stop=(kj == KD - 1))
            mu = ga.tile([1, 512], F32R, tag="gR")
            nc.vector.tensor_scalar(mu[:], sp[:], 1.0 / DIM, None, OP.mult)
            mu_pl = psy.tile([P, 512], F32, tag="rowA")
            nc.tensor.matmul(mu_pl[:], ones_row[:], mu[:], start=True, stop=True)
            mq = ga.tile([1, 512], F32, tag="gR")
            nc.vector.tensor_tensor(mq[:], mu[:], mu[:], OP.mult)
            vt = ga.tile([1, 512], F32, tag="gR")
            nc.vector.scalar_tensor_tensor(vt[:], qp[:], 1.0 / DIM, mq[:],
                                           OP.mult, OP.subtract)
            nc.scalar.activation(vt[:], vt[:], AF.Sqrt, bias=eps_sb[:, 1:2])
            rs = ga.tile([1, 512], F32R, tag="gR")
            nc.vector.reciprocal(rs[:], vt[:])
            rs_pl = psy.tile([P, 512], F32, tag="rowB")
            nc.tensor.matmul(rs_pl[:], ones_row[:], rs[:], start=True, stop=True)
            rs_sb = ga.tile([P, 512], F32, tag="gA")
            nc.scalar.activation(rs_sb[:], rs_pl[:], AF.Copy)
            for kj in range(KD):
                t1 = ga.tile([P, 512], F32, tag="gA")
                nc.vector.tensor_tensor(t1[:], hsm_sb[:, kj, fs], mu_pl[:],
                                        OP.subtract)
                nc.gpsimd.tensor_tensor(t1[:], t1[:], rs_sb[:], OP.mult)
                nc.vector.tensor_scalar(h2_bf[:, kj, fs], t1[:],
                                        a1[:, 2 * KD + kj:2 * KD + kj + 1],
                                        sh_mlp[:, kj:kj + 1], OP.mult, OP.add)

        # ---- MLP ----
        hmlp = sing.tile([P, KH, TH], BF16, tag="BIG8")
        for mj in range(KH):
            wf1 = ws.tile([P, KD, P], BF16, tag="ws")
            nc.sync.dma_start(wf1[:], w_fc1[:, mj * P:(mj + 1) * P]
                              .rearrange("(k p) m -> p k m", p=P))
            for ft in range(NTH):
                fs = slice(ft * 512, ft * 512 + 512)
                pm = ps.tile([P, 512], F32, tag="mm")
                for kj in range(KD):
                    nc.tensor.matmul(pm[:], wf1[:, kj], h2_bf[:, kj, fs],
                                     start=(kj == 0), stop=(kj == KD - 1))
                nc.scalar.activation(hmlp[:, mj, fs], pm[:], AF.Gelu_apprx_tanh,
                                     bias=fb_sb[:, mj:mj + 1])
        for oj in range(KD):
            wf2a = ws.tile([P, KH // 2, P], BF16, tag="ws")
            nc.sync.dma_start(wf2a[:], w_fc2[0:MLP_HID // 2, oj * P:(oj + 1) * P]
                              .rearrange("(k p) m -> p k m", p=P))
            wf2b = ws.tile([P, KH // 2, P], BF16, tag="ws")
            nc.sync.dma_start(wf2b[:], w_fc2[MLP_HID // 2:, oj * P:(oj + 1) * P]
                              .rearrange("(k p) m -> p k m", p=P))
            for ft in range(NTH):
                fs = slice(ft * 512, ft * 512 + 512)
                pf = ps.tile([P, 512], F32, tag="mm")
                for kj in range(KH):
                    wf2 = wf2a if kj < KH // 2 else wf2b
                    nc.tensor.matmul(pf[:], wf2[:, kj % (KH // 2)], hmlp[:, kj, fs],
                                     start=(kj == 0), stop=(kj == KH - 1))
                t1 = ga.tile([P, 512], F32, tag="gA")
                nc.vector.tensor_scalar(t1[:], pf[:], fb_sb[:, KH + oj:KH + oj + 1],
                                        gt_mlp[:, oj:oj + 1], OP.add, OP.mult)
                nc.vector.tensor_tensor(t1[:], t1[:], hsm_sb[:, oj, fs], OP.add)
                nc.sync.dma_start(
                    out_hs[:, :].rearrange("(k p) t -> p k t", p=P)[:, oj, fs], t1[:])
        ctx.close()
    nc.compile()
    return nc


def _prep_inputs(kw):
    f32 = np.float32
    bf16 = ml_dtypes.bfloat16
    g = lambda k: np.asarray(kw[k], f32)
    hs, cond = g("hidden_states"), g("cond")
    in_w = g("in_proj_w")
    A = -np.exp(g("A_log"))
    out_w = g("out_proj_w")

    def colchunks(v):
        return np.ascontiguousarray(v.reshape(-1, P).T)

    adaT = np.ascontiguousarray(g("ada_w").T).astype(bf16)
    fc1T = np.ascontiguousarray(g("fc1_w").T).astype(bf16)
    fc2T = np.ascontiguousarray(g("fc2_w").T).astype(bf16)
    in_maps = []
    for c in range(NC):
        b, i = c // 2, c % 2
        dsl = slice(i * DH, (i + 1) * DH)
        osl = slice((1 - i) * DH, (2 - i) * DH)
        A_h = A[dsl]
        A_pack = np.zeros((P, KDH * D_STATE), f32)
        for mj in range(KDH):
            A_pack[:, mj * D_STATE:(mj + 1) * D_STATE] = A_h[mj * P:(mj + 1) * P]
        psel = np.zeros((P, 2), f32)
        psel[:, i] = 1.0
        wout_rows = np.concatenate([out_w[:, dsl], out_w[:, osl]], axis=1)
        cw = g("conv_w")[dsl, 0, :].reshape(KDH, P, D_CONV).transpose(1, 0, 2)
        in_maps.append({
            "hsT": np.ascontiguousarray(hs[b].T).astype(bf16),
            "cond_c": colchunks(cond[b]),
            "w_ada": adaT,
            "ada_b": g("ada_b").reshape(1, -1),
            "nrm_w": colchunks(g("norm_w")),
            "nrm_b": colchunks(g("norm_b")),
            "w_inx": np.ascontiguousarray(in_w[dsl].T).astype(bf16),
            "w_inz": np.ascontiguousarray(in_w[D_INNER:][dsl].T).astype(bf16),
            "conv_wt": np.ascontiguousarray(cw.reshape(P, KDH * D_CONV)),
            "conv_bt": colchunks(g("conv_b")[dsl]),
            "w_xp": np.ascontiguousarray(g("x_proj_w")[:, dsl].T).astype(bf16),
            "w_dt": np.concatenate([g("dt_proj_w")[dsl].T,
                                    np.zeros((P - DT_RANK, DH), f32)], 0),
            "dt_bt": colchunks(g("dt_proj_b")[dsl]),
            "A_sc": A_pack,
            "Dp_c": colchunks(g("Dp")[dsl]),
            "w_out": np.ascontiguousarray(wout_rows.T).astype(bf16),
            "w_fc1": fc1T,
            "fc1_bt": colchunks(g("fc1_b")),
            "w_fc2": fc2T,
            "fc2_bt": colchunks(g("fc2_b")),
            "pairsel": psel,
        })
    return in_maps


def kernel(**inputs):
    if "nc" not in _CACHE:
        _CACHE["nc"] = _build()
    nc = _CACHE["nc"]
    in_maps = _prep_inputs(inputs)
    res = run_bass_kernel_spmd(nc, in_maps, list(range(NC)))
    hs_out = np.empty((B, L, DIM), np.float32)
    for c in range(NC):
        b, i = c // 2, c % 2
        hs_out[b, i * TH:(i + 1) * TH, :] = res.results[c]["out_hs"].T
    residual = np.asarray(inputs["hidden_states"], np.float32)
    return hs_out, residual
